# revision 34
# baseline (speedup 1.0000x reference)
"""CSR Linear kernel for TRN2: out = x @ W^T + bias, W from COO nonzeros.

Strategy: data-parallel over tokens across 8 NeuronCores. Host densifies the
sparse weight into WT[in, out] (duplicate coords summed), transposes x, and
casts both to bf16 (rel err ~2.4e-3, well under the 2e-2 gate; fp8 measured
3.8e-2 — fails). Each core computes its 1024-token shard with a tiled bf16
matmul accumulating in f32 PSUM: WT streamed from HBM once, x^T resident in
SBUF, bias fused into the PSUM->SBUF eviction, output stored bf16 and upcast
on host. bf16 (vs the earlier f32r) halves every DMA stream — killing the
~33us startup stall — and enables FWL fast weight loads on the PE.
"""

import os
import sys
import types

import numpy as np

TOKENS = 8192
IN_F = 4096
OUT_F = 4096
N_CORES = 8
P = 128

_CACHE = {}


def _ensure_ntff_hook():
    """Register the axon NTFF profile hook if the antenv stub lacks it.

    Only needed when tracing (BASS_TRACE=1); harmless otherwise. In
    environments with a real antenv.axon_hooks this is a no-op.
    """
    try:
        import antenv.axon_hooks  # noqa: F401

        return
    except ImportError:
        pass
    try:
        import antenv
        from trn_agent_boot.trn_boot import _ntff_profile_via_ctypes

        hooks = types.ModuleType("antenv.axon_hooks")
        hooks._hook = _ntff_profile_via_ctypes("/opt/axon/libaxon_pjrt.so")
        hooks.set_axon_ntff_profile_hook = lambda h: setattr(hooks, "_hook", h)
        hooks.get_axon_ntff_profile_hook = lambda: hooks._hook
        sys.modules["antenv.axon_hooks"] = hooks
        antenv.axon_hooks = hooks
    except Exception:
        pass


def _patch_upload():
    """Make trace artifact upload fall back to the local tmpdir when no
    artifact bucket is reachable (container environments)."""
    from concourse import bass_utils

    orig = bass_utils.upload_artifacts
    if getattr(orig, "_kernel_patched", False):
        return

    def _safe_upload(tmpdir):
        try:
            return orig(tmpdir)
        except Exception:
            return tmpdir

    _safe_upload._kernel_patched = True
    bass_utils.upload_artifacts = _safe_upload


def build_program(tok_per_core=TOKENS // N_CORES, in_f=IN_F, out_f=OUT_F):
    """Build + compile the per-core Bass program.

    out[tok_per_core, out_f] = xt.T @ wt + bias (all bf16 operands, f32 PSUM
    accumulate), with host-permuted layouts:
      xt [P, KO*tok_per_core]  xt[p, ko*T+t] = x[t, ko*128+p]
      wt [P, NB*KO*N_TILE]     wt[p, ((n*KO)+ko)*512+o] = W^T[ko*128+p, n*512+o]
      biasr [P, out_f]         host-replicated bias rows
    so every DMA line is long and per-partition contiguous.
    """
    key = (tok_per_core, in_f, out_f)
    if key in _CACHE:
        return _CACHE[key]

    import concourse.bacc as bacc
    import concourse.mybir as mybir
    import concourse.tile as tile

    N_TILE = 512  # out-feature block per psum bank
    KO = in_f // P  # k tiles
    M = tok_per_core // P  # token tiles
    NB = out_f // N_TILE  # out-feature blocks
    KO_CHUNK = 16  # k-tiles per WT DMA (2 MiB bf16 transfers)

    nc = bacc.Bacc("TRN2", target_bir_lowering=False, debug=False)

    # Host pre-permutes xt/wt so every DMA line is long and per-partition
    # contiguous (xt: 64 KiB/partition, wt: KO_CHUNK*N_TILE*2 = 8 KiB per
    # chunk) — the naive (ko p)/(mo p) layouts gave only 1-2 KiB lines and
    # ~130 GB/s effective DMA, stalling the PE all through startup.
    xt = nc.dram_tensor(
        "xt", [P, KO * tok_per_core], mybir.dt.bfloat16, kind="ExternalInput"
    )
    wt = nc.dram_tensor(
        "wt", [P, NB * KO * N_TILE], mybir.dt.bfloat16, kind="ExternalInput"
    )
    biasr = nc.dram_tensor("biasr", [P, out_f], mybir.dt.float32, kind="ExternalInput")
    out = nc.dram_tensor("out", [tok_per_core, out_f], mybir.dt.bfloat16, kind="ExternalOutput")

    xt_ap = xt.ap().rearrange("p (ko t) -> p ko t", ko=KO)  # [P, KO, T]
    wt_ap = wt.ap().rearrange("p (n ko o) -> p n ko o", n=NB, ko=KO)  # [P, NB, KO, N_TILE]
    out_ap = out.ap().rearrange("(mo p) o -> p mo o", p=P)  # [P, M, out_f]

    with tile.TileContext(nc) as tc:
        WT_BUFS = 6
        with (
            tc.tile_pool(name="xt_pool", bufs=1) as xt_pool,
            tc.tile_pool(name="bias_pool", bufs=1) as bias_pool,
            tc.tile_pool(name="wt_pool", bufs=WT_BUFS) as wt_pool,
            tc.tile_pool(name="out_pool", bufs=4) as out_pool,
            tc.tile_pool(name="psum", bufs=8, space="PSUM") as psum_pool,
        ):
            xt_sb = xt_pool.tile([P, KO, tok_per_core], mybir.dt.bfloat16)

            def bounds(ramp, step):
                b = [0]
                for r in ramp:
                    if b[-1] + r >= KO:
                        break
                    b.append(b[-1] + r)
                while b[-1] + step < KO:
                    b.append(b[-1] + step)
                b.append(KO)
                return list(zip(b[:-1], b[1:]))

            # Exponentially ramped leading chunks: the first matmul can start
            # after ~384 KiB of DMA, and each chunk's (cold) compute time
            # covers the DMA of the next — the DMA queues deliver slowly for
            # the first ~20us, so equal-size leading chunks stall the PE.
            wt_chunks = {
                n: bounds([1, 2, 4, 9] if n == 0 else [], KO_CHUNK)
                for n in range(NB)
            }
            xt_chunks = bounds([1, 2, 4], 4)

            def load_wt(n, kb, kbe):
                wt_t = wt_pool.tile(
                    [P, KO_CHUNK, N_TILE],
                    mybir.dt.bfloat16,
                    name=f"wt_{n}_{kb}",
                    tag="wt",
                )
                nc.sync.dma_start(wt_t[:, : kbe - kb, :], wt_ap[:, n, kb:kbe, :])
                return wt_t

            def load_xt(j, je):
                return nc.sync.dma_start(xt_sb[:, j:je, :], xt_ap[:, j:je, :])

            # Warm the PE clock-gate (HAM) during the dead window between
            # engine boot (~8.5us) and first-data arrival (~11.4us): the HAM
            # un-throttles 1.2->2.4 GHz only after ~3.4us of sustained PE
            # activity, so without this the first ~16 real matmuls run at
            # half clock. Results go to a scratch psum bank, discarded; the
            # only dependency is a gpsimd memset that lands well before the
            # PE queue drains its boot preamble.
            warm = bias_pool.tile([P, N_TILE], mybir.dt.bfloat16)
            nc.gpsimd.memset(warm[:], 1.0)
            warm_ps = psum_pool.tile(
                [P, N_TILE], mybir.dt.float32, name="warm_ps", tag="ps"
            )
            for _ in range(8):
                nc.tensor.matmul(
                    warm_ps[:], lhsT=warm[:, 0:P], rhs=warm[:], start=True, stop=True
                )

            bias_sb = bias_pool.tile([P, out_f], mybir.dt.float32)

            # Queue order: after the wt chunk covering k-tiles [kb, kbe), pull
            # the xt stream ahead to cover k-tile kbe — the PE needs (wt, xt)
            # pairs k-tile by k-tile. The bias arrives host-replicated as
            # [128, out_f] via one plain 2 MiB DMA (16 KiB/partition
            # contiguous), queued right after the last block-0 wt chunk: the
            # DMA engines ramp slowly for the first ~25us and anything queued
            # ahead of a k-chunk the PE is about to need stalls it; here it
            # still beats the first eviction's use of bias_sb by ~9us, and it
            # keeps ALL bias work off the strict-order PE queue (PE-matmul
            # broadcast variants serialized the PE behind bias DMAs stuck at
            # the back of the wt/xt stream — ~30us of startup stalls; a
            # gpsimd partition_broadcast crashed the ucode).
            preloaded = {}
            xi = 0
            bias_emitted = False
            for ci, (kb, kbe) in enumerate(wt_chunks[0][:WT_BUFS]):
                preloaded[(0, kb)] = load_wt(0, kb, kbe)
                if ci == len(wt_chunks[0]) - 1:
                    nc.sync.dma_start(bias_sb[:], biasr.ap())
                    bias_emitted = True
                while xi < len(xt_chunks) and xt_chunks[xi][0] <= min(kbe, KO - 1):
                    load_xt(*xt_chunks[xi])
                    xi += 1
            if not bias_emitted:
                nc.sync.dma_start(bias_sb[:], biasr.ap())
            if len(wt_chunks[0]) < WT_BUFS:
                kb, kbe = wt_chunks[1][0]
                preloaded[(1, kb)] = load_wt(1, kb, kbe)
            for j, je in xt_chunks[xi:]:
                load_xt(j, je)

            for n in range(NB):
                ns = slice(n * N_TILE, (n + 1) * N_TILE)
                ps = [
                    psum_pool.tile(
                        [P, N_TILE], mybir.dt.float32, name=f"ps_{n}_{m}", tag="ps"
                    )
                    for m in range(M)
                ]

                def evict(m):
                    ot = out_pool.tile(
                        [P, N_TILE], mybir.dt.bfloat16, name=f"ot_{n}_{m}", tag="ot"
                    )
                    nc.vector.tensor_add(out=ot[:], in0=ps[m][:], in1=bias_sb[:, ns])
                    nc.sync.dma_start(out_ap[:, m, ns], ot[:])

                for kb, kbe in wt_chunks[n]:
                    wt_t = preloaded.pop((n, kb), None)
                    if wt_t is None:
                        wt_t = load_wt(n, kb, kbe)
                    last_chunk = kbe == KO
                    if not last_chunk:
                        for kk in range(kbe - kb):
                            ko = kb + kk
                            for m in range(M):
                                nc.tensor.matmul(
                                    ps[m][:],
                                    lhsT=xt_sb[:, ko, m * P : (m + 1) * P],
                                    rhs=wt_t[:, kk, :],
                                    start=(ko == 0),
                                    stop=False,
                                )
                    else:
                        # m-outer on the final chunk so tiles finish staggered
                        # ~1.7us apart: each eviction (DVE + out-DMA) overlaps
                        # the next tile's matmuls instead of queuing in a
                        # serial burst after the block's last matmul.
                        for m in range(M):
                            for kk in range(kbe - kb):
                                ko = kb + kk
                                nc.tensor.matmul(
                                    ps[m][:],
                                    lhsT=xt_sb[:, ko, m * P : (m + 1) * P],
                                    rhs=wt_t[:, kk, :],
                                    start=(ko == 0),
                                    stop=(ko == KO - 1),
                                )
                            evict(m)

    nc.compile()
    _CACHE[key] = nc
    return nc


def _densify_wt(values, row_ids, col_ids, in_f=IN_F, out_f=OUT_F):
    """WT[i, o] = sum of values[k] over k with col_ids[k]==i, row_ids[k]==o."""
    idx = col_ids.astype(np.int64) * out_f + row_ids.astype(np.int64)
    wt = np.bincount(idx, weights=values.astype(np.float64), minlength=in_f * out_f)
    return np.ascontiguousarray(wt.astype(np.float32).reshape(in_f, out_f))


def kernel(x, values, row_ids, col_ids, bias):
    import concourse.mybir as mybir
    from concourse import bass_utils

    if os.environ.get("BASS_TRACE"):
        _ensure_ntff_hook()
        _patch_upload()

    nc = build_program()
    bf16 = mybir.dt.np(mybir.dt.bfloat16)

    x = np.asarray(x, dtype=np.float32)
    values = np.asarray(values, dtype=np.float32)
    row_ids = np.asarray(row_ids)
    col_ids = np.asarray(col_ids)
    bias = np.asarray(bias, dtype=np.float32)

    KO = IN_F // P
    N_TILE = 512
    NB = OUT_F // N_TILE
    tpc = TOKENS // N_CORES

    # wt_dev[p, n, ko, o'] = WT[ko*128 + p, n*512 + o'] — one contiguous
    # 8 KiB line per (partition, n, k-chunk) DMA read.
    wt = _densify_wt(values, row_ids, col_ids).astype(bf16)
    wt_dev = np.ascontiguousarray(
        wt.reshape(KO, P, NB, N_TILE).transpose(1, 2, 0, 3).reshape(P, -1)
    )
    bias_rep = np.ascontiguousarray(
        np.broadcast_to(bias.astype(np.float32)[None, :], (P, OUT_F))
    )
    in_maps = []
    for c in range(N_CORES):
        # xt_dev[p, ko, t] = x[c*tpc + t, ko*128 + p] — 64 KiB per partition.
        xs = x[c * tpc : (c + 1) * tpc, :].astype(bf16)
        xt_c = np.ascontiguousarray(
            xs.T.reshape(KO, P, tpc).transpose(1, 0, 2).reshape(P, -1)
        )
        in_maps.append({"xt": xt_c, "wt": wt_dev, "biasr": bias_rep})

    res = bass_utils.run_bass_kernel_spmd(nc, in_maps, core_ids=list(range(N_CORES)))
    global last_results
    last_results = res
    return np.concatenate(
        [res.results[c]["out"].astype(np.float32) for c in range(N_CORES)], axis=0
    )


last_results = None



# revision 35
# speedup vs baseline: 1.0054x; 1.0054x over previous
"""CSR Linear kernel for TRN2: out = x @ W^T + bias, W from COO nonzeros.

Strategy: data-parallel over tokens across 8 NeuronCores. Host densifies the
sparse weight into WT[in, out] (duplicate coords summed), transposes x, and
casts both to bf16 (rel err ~2.4e-3, well under the 2e-2 gate; fp8 measured
3.8e-2 — fails). Each core computes its 1024-token shard with a tiled bf16
matmul accumulating in f32 PSUM: WT streamed from HBM once, x^T resident in
SBUF, bias fused into the PSUM->SBUF eviction, output stored bf16 and upcast
on host. bf16 (vs the earlier f32r) halves every DMA stream — killing the
~33us startup stall — and enables FWL fast weight loads on the PE.
"""

import os
import sys
import types

import numpy as np

TOKENS = 8192
IN_F = 4096
OUT_F = 4096
N_CORES = 8
P = 128

_CACHE = {}


def _ensure_ntff_hook():
    """Register the axon NTFF profile hook if the antenv stub lacks it.

    Only needed when tracing (BASS_TRACE=1); harmless otherwise. In
    environments with a real antenv.axon_hooks this is a no-op.
    """
    try:
        import antenv.axon_hooks  # noqa: F401

        return
    except ImportError:
        pass
    try:
        import antenv
        from trn_agent_boot.trn_boot import _ntff_profile_via_ctypes

        hooks = types.ModuleType("antenv.axon_hooks")
        hooks._hook = _ntff_profile_via_ctypes("/opt/axon/libaxon_pjrt.so")
        hooks.set_axon_ntff_profile_hook = lambda h: setattr(hooks, "_hook", h)
        hooks.get_axon_ntff_profile_hook = lambda: hooks._hook
        sys.modules["antenv.axon_hooks"] = hooks
        antenv.axon_hooks = hooks
    except Exception:
        pass


def _patch_upload():
    """Make trace artifact upload fall back to the local tmpdir when no
    artifact bucket is reachable (container environments)."""
    from concourse import bass_utils

    orig = bass_utils.upload_artifacts
    if getattr(orig, "_kernel_patched", False):
        return

    def _safe_upload(tmpdir):
        try:
            return orig(tmpdir)
        except Exception:
            return tmpdir

    _safe_upload._kernel_patched = True
    bass_utils.upload_artifacts = _safe_upload


def build_program(tok_per_core=TOKENS // N_CORES, in_f=IN_F, out_f=OUT_F):
    """Build + compile the per-core Bass program.

    out[tok_per_core, out_f] = xt.T @ wt + bias (all bf16 operands, f32 PSUM
    accumulate), with host-permuted layouts:
      xt [P, KO*tok_per_core]  xt[p, ko*T+t] = x[t, ko*128+p]
      wt [P, NB*KO*N_TILE]     wt[p, ((n*KO)+ko)*512+o] = W^T[ko*128+p, n*512+o]
      biasr [P, out_f]         host-replicated bias rows
    so every DMA line is long and per-partition contiguous.
    """
    key = (tok_per_core, in_f, out_f)
    if key in _CACHE:
        return _CACHE[key]

    import concourse.bacc as bacc
    import concourse.mybir as mybir
    import concourse.tile as tile

    N_TILE = 512  # out-feature block per psum bank
    KO = in_f // P  # k tiles
    M = tok_per_core // P  # token tiles
    NB = out_f // N_TILE  # out-feature blocks
    KO_CHUNK = 16  # k-tiles per WT DMA (2 MiB bf16 transfers)

    nc = bacc.Bacc("TRN2", target_bir_lowering=False, debug=False)

    # Host pre-permutes xt/wt so every DMA line is long and per-partition
    # contiguous (xt: 64 KiB/partition, wt: KO_CHUNK*N_TILE*2 = 8 KiB per
    # chunk) — the naive (ko p)/(mo p) layouts gave only 1-2 KiB lines and
    # ~130 GB/s effective DMA, stalling the PE all through startup.
    xt = nc.dram_tensor(
        "xt", [P, KO * tok_per_core], mybir.dt.bfloat16, kind="ExternalInput"
    )
    wt = nc.dram_tensor(
        "wt", [P, NB * KO * N_TILE], mybir.dt.bfloat16, kind="ExternalInput"
    )
    biasr = nc.dram_tensor("biasr", [P, out_f], mybir.dt.float32, kind="ExternalInput")
    out = nc.dram_tensor("out", [tok_per_core, out_f], mybir.dt.bfloat16, kind="ExternalOutput")

    xt_ap = xt.ap().rearrange("p (ko t) -> p ko t", ko=KO)  # [P, KO, T]
    wt_ap = wt.ap().rearrange("p (n ko o) -> p n ko o", n=NB, ko=KO)  # [P, NB, KO, N_TILE]
    out_ap = out.ap().rearrange("(mo p) o -> p mo o", p=P)  # [P, M, out_f]

    with tile.TileContext(nc) as tc:
        WT_BUFS = 6
        with (
            tc.tile_pool(name="xt_pool", bufs=1) as xt_pool,
            tc.tile_pool(name="bias_pool", bufs=1) as bias_pool,
            tc.tile_pool(name="wt_pool", bufs=WT_BUFS) as wt_pool,
            tc.tile_pool(name="out_pool", bufs=4) as out_pool,
            tc.tile_pool(name="psum", bufs=8, space="PSUM") as psum_pool,
        ):
            xt_sb = xt_pool.tile([P, KO, tok_per_core], mybir.dt.bfloat16)

            def bounds(ramp, step):
                b = [0]
                for r in ramp:
                    if b[-1] + r >= KO:
                        break
                    b.append(b[-1] + r)
                while b[-1] + step < KO:
                    b.append(b[-1] + step)
                b.append(KO)
                return list(zip(b[:-1], b[1:]))

            # Exponentially ramped leading chunks: the first matmul can start
            # after ~384 KiB of DMA, and each chunk's (cold) compute time
            # covers the DMA of the next — the DMA queues deliver slowly for
            # the first ~20us, so equal-size leading chunks stall the PE.
            wt_chunks = {
                n: bounds([1, 2, 4, 9] if n == 0 else [], KO_CHUNK)
                for n in range(NB)
            }
            xt_chunks = bounds([1, 2, 4], 4)

            def load_wt(n, kb, kbe):
                wt_t = wt_pool.tile(
                    [P, KO_CHUNK, N_TILE],
                    mybir.dt.bfloat16,
                    name=f"wt_{n}_{kb}",
                    tag="wt",
                )
                nc.sync.dma_start(wt_t[:, : kbe - kb, :], wt_ap[:, n, kb:kbe, :])
                return wt_t

            def load_xt(j, je):
                return nc.sync.dma_start(xt_sb[:, j:je, :], xt_ap[:, j:je, :])

            # (A PE warm-up variant — dummy matmuls during the boot window to
            # pre-flip the HAM clock-gate before first data arrives — was
            # measured net-negative at +3.5us: the ~2us idle gap between the
            # dummies finishing and the first wt/xt chunks landing lets the
            # HAM re-throttle, so the dummies only add their own runtime.)
            bias_sb = bias_pool.tile([P, out_f], mybir.dt.float32)

            # Queue order: after the wt chunk covering k-tiles [kb, kbe), pull
            # the xt stream ahead to cover k-tile kbe — the PE needs (wt, xt)
            # pairs k-tile by k-tile. The bias arrives host-replicated as
            # [128, out_f] via one plain 2 MiB DMA (16 KiB/partition
            # contiguous), queued right after the last block-0 wt chunk: the
            # DMA engines ramp slowly for the first ~25us and anything queued
            # ahead of a k-chunk the PE is about to need stalls it; here it
            # still beats the first eviction's use of bias_sb by ~9us, and it
            # keeps ALL bias work off the strict-order PE queue (PE-matmul
            # broadcast variants serialized the PE behind bias DMAs stuck at
            # the back of the wt/xt stream — ~30us of startup stalls; a
            # gpsimd partition_broadcast crashed the ucode).
            preloaded = {}
            xi = 0
            bias_emitted = False
            for ci, (kb, kbe) in enumerate(wt_chunks[0][:WT_BUFS]):
                preloaded[(0, kb)] = load_wt(0, kb, kbe)
                if ci == len(wt_chunks[0]) - 1:
                    nc.sync.dma_start(bias_sb[:], biasr.ap())
                    bias_emitted = True
                while xi < len(xt_chunks) and xt_chunks[xi][0] <= min(kbe, KO - 1):
                    load_xt(*xt_chunks[xi])
                    xi += 1
            if not bias_emitted:
                nc.sync.dma_start(bias_sb[:], biasr.ap())
            if len(wt_chunks[0]) < WT_BUFS:
                kb, kbe = wt_chunks[1][0]
                preloaded[(1, kb)] = load_wt(1, kb, kbe)
            for j, je in xt_chunks[xi:]:
                load_xt(j, je)

            for n in range(NB):
                ns = slice(n * N_TILE, (n + 1) * N_TILE)
                ps = [
                    psum_pool.tile(
                        [P, N_TILE], mybir.dt.float32, name=f"ps_{n}_{m}", tag="ps"
                    )
                    for m in range(M)
                ]

                def evict(m):
                    ot = out_pool.tile(
                        [P, N_TILE], mybir.dt.bfloat16, name=f"ot_{n}_{m}", tag="ot"
                    )
                    nc.vector.tensor_add(out=ot[:], in0=ps[m][:], in1=bias_sb[:, ns])
                    nc.sync.dma_start(out_ap[:, m, ns], ot[:])

                for kb, kbe in wt_chunks[n]:
                    wt_t = preloaded.pop((n, kb), None)
                    if wt_t is None:
                        wt_t = load_wt(n, kb, kbe)
                    last_chunk = kbe == KO
                    if not last_chunk:
                        for kk in range(kbe - kb):
                            ko = kb + kk
                            for m in range(M):
                                nc.tensor.matmul(
                                    ps[m][:],
                                    lhsT=xt_sb[:, ko, m * P : (m + 1) * P],
                                    rhs=wt_t[:, kk, :],
                                    start=(ko == 0),
                                    stop=False,
                                )
                    else:
                        # m-outer on the final chunk so tiles finish staggered
                        # ~1.7us apart: each eviction (DVE + out-DMA) overlaps
                        # the next tile's matmuls instead of queuing in a
                        # serial burst after the block's last matmul.
                        for m in range(M):
                            for kk in range(kbe - kb):
                                ko = kb + kk
                                nc.tensor.matmul(
                                    ps[m][:],
                                    lhsT=xt_sb[:, ko, m * P : (m + 1) * P],
                                    rhs=wt_t[:, kk, :],
                                    start=(ko == 0),
                                    stop=(ko == KO - 1),
                                )
                            evict(m)

    nc.compile()
    _CACHE[key] = nc
    return nc


def _densify_wt(values, row_ids, col_ids, in_f=IN_F, out_f=OUT_F):
    """WT[i, o] = sum of values[k] over k with col_ids[k]==i, row_ids[k]==o."""
    idx = col_ids.astype(np.int64) * out_f + row_ids.astype(np.int64)
    wt = np.bincount(idx, weights=values.astype(np.float64), minlength=in_f * out_f)
    return np.ascontiguousarray(wt.astype(np.float32).reshape(in_f, out_f))


def kernel(x, values, row_ids, col_ids, bias):
    import concourse.mybir as mybir
    from concourse import bass_utils

    if os.environ.get("BASS_TRACE"):
        _ensure_ntff_hook()
        _patch_upload()

    nc = build_program()
    bf16 = mybir.dt.np(mybir.dt.bfloat16)

    x = np.asarray(x, dtype=np.float32)
    values = np.asarray(values, dtype=np.float32)
    row_ids = np.asarray(row_ids)
    col_ids = np.asarray(col_ids)
    bias = np.asarray(bias, dtype=np.float32)

    KO = IN_F // P
    N_TILE = 512
    NB = OUT_F // N_TILE
    tpc = TOKENS // N_CORES

    # wt_dev[p, n, ko, o'] = WT[ko*128 + p, n*512 + o'] — one contiguous
    # 8 KiB line per (partition, n, k-chunk) DMA read.
    wt = _densify_wt(values, row_ids, col_ids).astype(bf16)
    wt_dev = np.ascontiguousarray(
        wt.reshape(KO, P, NB, N_TILE).transpose(1, 2, 0, 3).reshape(P, -1)
    )
    bias_rep = np.ascontiguousarray(
        np.broadcast_to(bias.astype(np.float32)[None, :], (P, OUT_F))
    )
    in_maps = []
    for c in range(N_CORES):
        # xt_dev[p, ko, t] = x[c*tpc + t, ko*128 + p] — 64 KiB per partition.
        xs = x[c * tpc : (c + 1) * tpc, :].astype(bf16)
        xt_c = np.ascontiguousarray(
            xs.T.reshape(KO, P, tpc).transpose(1, 0, 2).reshape(P, -1)
        )
        in_maps.append({"xt": xt_c, "wt": wt_dev, "biasr": bias_rep})

    res = bass_utils.run_bass_kernel_spmd(nc, in_maps, core_ids=list(range(N_CORES)))
    global last_results
    last_results = res
    return np.concatenate(
        [res.results[c]["out"].astype(np.float32) for c in range(N_CORES)], axis=0
    )


last_results = None

